# revision 33
# baseline (speedup 1.0000x reference)
"""GQA attention (B=4, L=1024, D=4096, 32 Q heads / 8 KV heads, head_dim=128,
traditional RoPE, causal mask) on 8 TRN2 NeuronCores.

Sharding: tensor-parallel over heads. Core c owns Q heads {c, c+8, c+16, c+24}
(all map to KV head c) - each core needs exactly one KV head. wq/wk/wv
column-sharded, wo row-sharded, x replicated. Each core computes a partial
output through wo; the host sums the 8 partials (and transposes: the kernel
writes out^T [DIM, T] in bf16).

v3 changes vs v2 (measured ~697-700us vs ~706-712us baseline, trace on):
- Attention uses 128-token q chunks x 4-head-wide score/PV matmuls (N=512).
  Finer causal blocking: 36 instead of 40 block-columns per batch. The
  softmax denominator comes from per-group pair-sums + a pairwise tree on
  DVE and ONE ones-matmul per q-chunk, cutting ~25k PE cycles net. (An
  earlier variant chained the sums on GpSimd - its serial ~1.2us/add chain
  starved the PE; keep this work on DVE, tree-shaped.)
- v^T produced by the DMA xbar transpose (dma_start_transpose) instead of PE
  matmul-transposes: no identity operand, no PSUM transpose bank.
- One unified 8-bank PSUM pool (4 slots x 2 banks, tags sc/pvden) shared by
  the QKV stage, attention and the output projection - no pool-boundary
  barriers between stages, slots rotate straight across A->B(b)->C(b)->B(b+1),
  and batch-0 attention is emitted directly behind the last QKV chunk.
- x is host-packed [p, tci, d, t] so each x DMA moves 4 d-slices with one
  contiguous 4KB segment per partition; prefetch runs a fixed 2 groups
  (8 d-slices) ahead - shallower prefetch measurably stalls stage A once
  per tci. wo streams in 4x1MB chunks (tci 1-4) so no single weight burst
  starves the x stream. RoPE q reuses the k cos/sin tables (same positional
  slices) with per-head muls, saving 20KB/partition of SBUF.
- Output projection writes alternate between the sync and scalar DMA rings,
  halving the end-of-kernel drain.
"""

import numpy as np
import ml_dtypes
from contextlib import ExitStack

import concourse.bass as bass
import concourse.mybir as mybir
import concourse.tile as tile
from concourse import bacc
from concourse.bass_utils import run_bass_kernel_spmd

DIM = 4096
N_HEADS = 32
N_KV = 8
DH = 128
B, L = 4, 1024
NCORES = 8
HPC = N_HEADS // NCORES  # 4 q-heads per core
T = B * L  # 4096 tokens total
SCALE = DH ** -0.5
ROPE_BASE = 10000.0
NDT = DIM // 128  # 32 contraction tiles

BF = mybir.dt.bfloat16
F32 = mybir.dt.float32
NPBF = ml_dtypes.bfloat16

QC = L // 128  # 8 q-chunks of 128 per batch
KT = L // 128  # 8 k tiles of 128 per batch
NTCI = T // 512  # 8 chunks of 512 tokens

TRACE = False
LAST_RESULT = [None]
DEBUG_STOP = None  # "B0" stops the build after batch-0 attention (sim debug)


def _check_mask(mask):
    """Verify the mask is the binary causal mask this kernel is specialized
    to."""
    m = np.asarray(mask)
    assert m.shape == (L, L)
    assert np.all((m == 0.0) | (m <= -1e8)), "kernel assumes binary additive mask"
    keep = (m == 0.0)  # [q, k]
    expect = np.tril(np.ones((L, L), dtype=bool))
    assert np.array_equal(keep, expect), "kernel assumes causal mask"


def _build():
    nc = bacc.Bacc(
        "TRN2", target_bir_lowering=False, debug=False, num_devices=NCORES
    )

    # x host-packed: [p, tci, d, t] so a 4-d-slice chunk is one contiguous
    # 4KB-per-partition DMA
    xt4 = nc.dram_tensor("xt4", [128, NTCI * NDT * 512], BF, kind="ExternalInput").ap()
    # all QKV weights in one tensor, d-major: [:, d] = [wv_d | wk_d | wq_d]
    wall = nc.dram_tensor("wall", [128, NDT * 768], BF, kind="ExternalInput").ap()
    wo = nc.dram_tensor("wo", [128, HPC * DIM], BF, kind="ExternalInput").ap()
    cosq = sinq = None
    cosk = nc.dram_tensor("cosk", [128, L], BF, kind="ExternalInput").ap()
    sink = nc.dram_tensor("sink", [128, L], BF, kind="ExternalInput").ap()
    dmask4 = nc.dram_tensor("dmask4", [128, HPC * 128], BF, kind="ExternalInput").ap()
    out = nc.dram_tensor("out", [DIM, T], BF, kind="ExternalOutput").ap()

    xt4_r = xt4.rearrange("p (tci d t) -> p tci d t", tci=NTCI, d=NDT)
    wall_r = wall.rearrange("p (dt m) -> p dt m", dt=NDT)  # [128, 32, 768]
    del cosq, sinq  # q rope shares the k tables (same positional slices)

    with tile.TileContext(nc) as tc, ExitStack() as ctx:
        persist = ctx.enter_context(tc.tile_pool(name="persist", bufs=1))
        qt_pool = ctx.enter_context(tc.tile_pool(name="qt", bufs=B))
        kt_pool = ctx.enter_context(tc.tile_pool(name="kt", bufs=B))
        v_pool = ctx.enter_context(tc.tile_pool(name="v", bufs=B))
        wo_p = ctx.enter_context(tc.tile_pool(name="wo_p", bufs=1))
        # unified PSUM pool: 4 slots x 2 banks, tags "sc" and "pvden"
        psU = ctx.enter_context(tc.tile_pool(name="psU", bufs=2, space="PSUM"))
        ones_sb = persist.tile([128, 128], BF)
        nc.vector.memset(ones_sb, 1.0)
        dmsk_sb = persist.tile([128, 128], BF)
        wo_sb = wo_p.tile([128, HPC, DIM], BF)

        qt_all = [None] * B  # [128 dh, HPC, 1024] rope'd q, halves layout
        kt_all = [None] * B  # [128 dh, 1024]
        v_t = [None] * B     # [128 t, KT, 128 dh]

        # ---------------- Stage A: QKV projection + RoPE ----------------
        with tc.tile_pool(name="wA", bufs=1) as wA, \
             tc.tile_pool(name="xp", bufs=3) as xp, \
             tc.tile_pool(name="evq", bufs=2) as evq, \
             tc.tile_pool(name="evs", bufs=2) as evs, \
             tc.tile_pool(name="rtmp", bufs=1) as rtmp:

            wall_sb = wA.tile([128, NDT, 768], BF)
            cosk_sb = wA.tile([128, L], BF)
            sink_sb = wA.tile([128, L], BF)

            def _wv(d):
                return wall_sb[:, d, 0:DH]

            def _wk(d):
                return wall_sb[:, d, DH:2 * DH]

            def _wq(d, h):
                return wall_sb[:, d, 2 * DH + h * DH:2 * DH + (h + 1) * DH]

            # weight chunks of 4 d-slices on the scalar ring, concurrent with
            # the x stream on the sync ring
            def _wchunk(k):
                dsl = slice(4 * k, 4 * k + 4)
                nc.scalar.dma_start(out=wall_sb[:, dsl], in_=wall_r[:, dsl])

            # x group g = (tci, j): 4 d-slices [128, 4, 512]
            xtiles = {}

            def _xfetch(g):
                tci_g, j = divmod(g, 8)
                xt = xp.tile([128, 4, 512], BF, name="xt4t")
                nc.sync.dma_start(
                    out=xt, in_=xt4_r[:, tci_g, 4 * j:4 * j + 4, :]
                )
                xtiles[g] = xt

            nc.scalar.dma_start(out=wall_sb[:, 0:1], in_=wall_r[:, 0:1])
            _xfetch(0)
            nc.scalar.dma_start(out=wall_sb[:, 1:4], in_=wall_r[:, 1:4])
            _xfetch(1)
            _wchunk(1)
            _xfetch(2)
            _wchunk(2)
            _wchunk(3)

            # HAM warmup: keep the PE busy while the first DMAs land
            wu = psU.tile([128, 2, 512], F32, name="wu", tag="sc")
            for _ in range(160):
                nc.tensor.matmul(
                    wu[:, 0, 0:128], ones_sb, ones_sb, start=True, stop=True
                )

            for tci in range(NTCI):
                b, half = tci // 2, tci % 2
                lsl = slice(half * 512, (half + 1) * 512)
                if half == 0:
                    qt_all[b] = qt_pool.tile([128, HPC, L], BF, name="qtile")
                    kt_all[b] = kt_pool.tile([128, L], BF, name="ktile")
                    v_t[b] = v_pool.tile([128, KT, DH], BF, name="vtile")

                vk = psU.tile([128, 2, 512], F32, name="vk", tag="sc")
                q01 = psU.tile([128, 2, 512], F32, name="q01", tag="sc")
                q23 = psU.tile([128, 2, 512], F32, name="q23", tag="pvden")

                for d in range(NDT):
                    if tci == 0:
                        if d % 4 == 1 and d // 4 + 4 < 8:
                            _wchunk(d // 4 + 4)
                        if d == 8:
                            nc.scalar.dma_start(
                                out=dmsk_sb, in_=dmask4[:, 0:128]
                            )
                        if d == 20:
                            nc.scalar.dma_start(out=cosk_sb, in_=cosk)
                        if d == 23:
                            nc.scalar.dma_start(out=sink_sb, in_=sink)
                    if 1 <= tci <= 4 and d == 16:
                        # wo in 4x1MB chunks so no burst starves the x stream
                        h = tci - 1
                        nc.scalar.dma_start(
                            out=wo_sb[:, h],
                            in_=wo[:, h * DIM:(h + 1) * DIM],
                        )
                    if d % 4 == 0:
                        g = tci * 8 + d // 4
                        if g + 2 < NTCI * 8 and g + 2 not in xtiles:
                            _xfetch(g + 2)
                        xtiles.pop(g - 1, None)  # free previous group ref
                        cur = xtiles[g]
                    xt = cur[:, d % 4]
                    st, sp = d == 0, d == NDT - 1
                    nc.tensor.matmul(vk[:, 0], _wv(d), xt, start=st, stop=sp)
                    nc.tensor.matmul(vk[:, 1], _wk(d), xt, start=st, stop=sp)
                    nc.tensor.matmul(q01[:, 0], _wq(d, 0), xt, start=st, stop=sp)
                    nc.tensor.matmul(q01[:, 1], _wq(d, 1), xt, start=st, stop=sp)
                    nc.tensor.matmul(q23[:, 0], _wq(d, 2), xt, start=st, stop=sp)
                    nc.tensor.matmul(q23[:, 1], _wq(d, 3), xt, start=st, stop=sp)

                # --- tail: evacuate + v-transpose (DMA xbar) + RoPE ---
                vraw = evs.tile([128, 512], BF, name="vraw")
                nc.scalar.copy(vraw, vk[:, 0])
                nc.sync.dma_start_transpose(
                    out=v_t[b][:, half * 4:(half + 1) * 4, :], in_=vraw
                )

                kraw = evs.tile([128, 512], BF, name="kraw")
                nc.vector.tensor_copy(kraw, vk[:, 1])
                qraw = evq.tile([128, HPC, 512], BF, name="qraw")
                nc.scalar.copy(qraw[:, 0:2], q01)
                nc.vector.tensor_copy(qraw[:, 2:4], q23)

                # RoPE: dst = raw*cos + swap64(raw*sinSw)  (halves layout)
                u_k = rtmp.tile([128, 512], BF, name="uk")
                t_k = rtmp.tile([128, 512], BF, name="tk")
                usw_k = rtmp.tile([128, 512], BF, name="uswk")
                nc.vector.tensor_mul(u_k, kraw, sink_sb[:, lsl])
                nc.vector.tensor_mul(t_k, kraw, cosk_sb[:, lsl])
                nc.vector.tensor_copy(usw_k[0:64], u_k[64:128])
                nc.vector.tensor_copy(usw_k[64:128], u_k[0:64])
                nc.vector.tensor_add(kt_all[b][:, lsl], t_k, usw_k)

                u_q = rtmp.tile([128, HPC, 512], BF, name="uq")
                t_q = rtmp.tile([128, HPC, 512], BF, name="tq")
                usw_q = rtmp.tile([128, HPC, 512], BF, name="uswq")
                for h in range(HPC):
                    nc.vector.tensor_mul(u_q[:, h], qraw[:, h], sink_sb[:, lsl])
                    nc.vector.tensor_mul(t_q[:, h], qraw[:, h], cosk_sb[:, lsl])
                nc.vector.tensor_copy(usw_q[0:64], u_q[64:128])
                nc.vector.tensor_copy(usw_q[64:128], u_q[0:64])
                nc.vector.tensor_add(qt_all[b][:, :, lsl], t_q, usw_q)

        # ---------------- Stages B + C, per batch ----------------
        attn_pool = ctx.enter_context(tc.tile_pool(name="attn", bufs=2))
        ep = ctx.enter_context(tc.tile_pool(name="ep", bufs=4))
        e2p = ctx.enter_context(tc.tile_pool(name="e2p", bufs=8))
        rcp = ctx.enter_context(tc.tile_pool(name="rcp", bufs=2))
        oev = ctx.enter_context(tc.tile_pool(name="oev", bufs=4))
        attn_all = [None] * B

        def gen_B(b):
            """Emit batch-b attention one q-chunk chain per next()."""
            attn_all[b] = attn_pool.tile([128, HPC, L], BF, name="atile")

            # ---- B(b): attention. 128-token q chunks, 4 heads wide.
            # Software-pipelined one score-group ahead: the PE always has the
            # next group's score matmuls queued while ACT runs exp and DVE
            # accumulates the softmax denominator.
            pvden_cur = [None]  # pv/den psum of the current q chunk
            e2s_cur = [[]]      # per-group pair-sums of e for the denominator
            pending = [None]

            def flush_pv(item):
                qc, g, e_g, first, last, e2s = item
                if first:
                    pvden_cur[0] = psU.tile(
                        [128, 2, 512], F32, name="pvden", tag="pvden"
                    )
                pvden = pvden_cur[0]
                for i, kt in enumerate(g):
                    nc.tensor.matmul(
                        pvden[:, 0], v_t[b][:, kt], e_g[:, i],
                        start=(first and i == 0), stop=(last and i == len(g) - 1),
                    )
                if last:
                    # DVE pairwise tree over the group sums, then one
                    # ones-matmul gives the softmax denominator
                    lvl = list(e2s)
                    while len(lvl) > 1:
                        nxt = []
                        for j in range(0, len(lvl) - 1, 2):
                            t_ = e2p.tile([128, 512], BF, name="e4")
                            nc.vector.tensor_add(t_, lvl[j], lvl[j + 1])
                            nxt.append(t_)
                        if len(lvl) % 2:
                            nxt.append(lvl[-1])
                        lvl = nxt
                    nc.tensor.matmul(
                        pvden[:, 1], ones_sb, lvl[0],
                        start=True, stop=True,
                    )
                    recip = rcp.tile([128, 512], F32, name="recip")
                    nc.vector.reciprocal_approx_fast(recip, pvden[:, 1])
                    q_sl = slice(qc * 128, (qc + 1) * 128)
                    nc.vector.tensor_mul(
                        attn_all[b][:, :, q_sl],
                        pvden[:, 0].rearrange("p (h q) -> p h q", h=HPC),
                        recip.rearrange("p (h q) -> p h q", h=HPC),
                    )

            for qc in reversed(range(QC)):
                kts = list(range(qc + 1))
                groups = [kts[i:i + 2] for i in range(0, len(kts), 2)]
                q_ap = qt_all[b][:, :, qc * 128:(qc + 1) * 128]  # [128,4,128]
                for gi, g in enumerate(groups):
                    s = len(g)
                    first, last = gi == 0, gi == len(groups) - 1
                    sc = psU.tile(
                        [128, 2, HPC, 128], F32, name="sc", tag="sc",
                    )
                    for i, kt in enumerate(g):
                        nc.tensor.matmul(
                            sc[:, i],
                            kt_all[b][:, kt * 128:(kt + 1) * 128],
                            q_ap,
                            start=True, stop=True,
                        )
                    e_g = ep.tile([128, 2, HPC, 128], BF, name="etile")
                    nc.scalar.activation(
                        e_g[:, 0:s], sc[:, 0:s],
                        mybir.ActivationFunctionType.Exp,
                        scale=SCALE,
                    )
                    if last:
                        # diagonal block: causal keep-pattern, same for all qc
                        for h_ in range(HPC):
                            nc.vector.tensor_mul(
                                e_g[:, s - 1, h_], e_g[:, s - 1, h_], dmsk_sb
                            )
                    # per-group pair-sum for the denominator (DVE, parallel)
                    if first:
                        e2s_cur[0] = []
                    if s == 2:
                        e2 = e2p.tile([128, 512], BF, name="e2")
                        nc.vector.tensor_add(
                            e2, e_g[:, 0].rearrange("p h q -> p (h q)"),
                            e_g[:, 1].rearrange("p h q -> p (h q)"),
                        )
                    else:
                        e2 = e_g[:, 0].rearrange("p h q -> p (h q)")
                    e2s_cur[0].append(e2)
                    if pending[0] is not None:
                        flush_pv(pending[0])
                    pending[0] = (qc, g, e_g, first, last, list(e2s_cur[0]))
                yield qc
            flush_pv(pending[0])
            pending[0] = None
            yield QC

        gens = [gen_B(b_) for b_ in range(B)]

        def pump(gen, n=None):
            took = 0
            for _ in gen:
                took += 1
                if n is not None and took >= n:
                    return

        pump(gens[0])  # B(0) in full, right on the heels of stage A

        if DEBUG_STOP != "B0":
            for b in range(B):
                # ---- C(b): output projection, wo stationary, out^T ----
                for nb in range(DIM // 128):  # 32 blocks of 128 output cols
                    ps_c = psU.tile(
                        [128, 2, 512], F32, name="psc",
                        tag=("sc" if nb % 2 == 0 else "pvden"),
                    )
                    for h in range(HPC):
                        for t2 in (1, 0):
                            nc.tensor.matmul(
                                ps_c[:, t2],
                                wo_sb[:, h, nb * 128:(nb + 1) * 128],
                                attn_all[b][:, h, t2 * 512:(t2 + 1) * 512],
                                start=(h == 0), stop=(h == HPC - 1),
                            )
                    o_sb = oev.tile([128, 1024], BF, name="osb")
                    if nb % 2 == 0:
                        nc.vector.tensor_copy(o_sb, ps_c)
                        eng = nc.sync
                    else:
                        nc.scalar.copy(o_sb, ps_c)
                        eng = nc.scalar
                    eng.dma_start(
                        out=out[nb * 128:(nb + 1) * 128, b * L:(b + 1) * L],
                        in_=o_sb,
                    )
                if b + 1 < B:
                    pump(gens[b + 1])  # rest of B(b+1)

    nc.finalize()
    return nc


def _host_tables():
    """cos/sin tables in the halves layout: row i (i<64) = even dim 2i,
    row 64+i = odd dim 2i+1. u = raw*sinSw; dst_lo = t1_lo + u_hi needs
    sinSw = [+sin; -sin]; cosH = [cos; cos]."""
    inv = ROPE_BASE ** (-np.arange(0, DH, 2, dtype=np.float64) / DH)  # [64]
    pos = np.arange(L, dtype=np.float64)
    ang = inv[:, None] * pos[None, :]  # [64, L]
    cosA, sinA = np.cos(ang), np.sin(ang)
    cosH = np.concatenate([cosA, cosA], axis=0)  # [128, L]
    sinSw = np.concatenate([sinA, -sinA], axis=0)  # [128, L]
    cosq = np.empty((128, 2, HPC, 512), dtype=np.float64)
    sinq = np.empty((128, 2, HPC, 512), dtype=np.float64)
    for half in range(2):
        sl = slice(half * 512, (half + 1) * 512)
        cosq[:, half] = cosH[:, sl][:, None, :]
        sinq[:, half] = sinSw[:, sl][:, None, :]
    return (
        cosq.reshape(128, -1).astype(NPBF),
        sinq.reshape(128, -1).astype(NPBF),
        np.ascontiguousarray(cosH).astype(NPBF),
        np.ascontiguousarray(sinSw).astype(NPBF),
    )


def _ptile(w):
    # [K, M] -> partition-major [128, (K/128, M)] host pre-tiling
    k, m = w.shape
    return np.ascontiguousarray(
        w.reshape(k // 128, 128, m).transpose(1, 0, 2).reshape(128, -1)
    ).astype(NPBF)


def _host_inputs(x, mask, wq, wk, wv, wo):
    _check_mask(mask)

    xT = np.ascontiguousarray(x.reshape(T, DIM).T).astype(NPBF)
    # packed [p, tci, d, t]
    xt4 = np.ascontiguousarray(
        xT.reshape(NDT, 128, NTCI, 512).transpose(1, 2, 0, 3).reshape(128, -1)
    )
    _, _, cosk, sink = _host_tables()
    # diagonal causal keep-pattern [k, q] = (q >= k), replicated x4 heads
    tri = np.triu(np.ones((128, 128), dtype=np.float32))
    dmask4 = np.ascontiguousarray(
        np.tile(tri[:, None, :], (1, HPC, 1)).reshape(128, -1)
    ).astype(NPBF)

    # halves permutation of the head_dim axis: even dims then odd dims
    perm = np.concatenate([np.arange(0, DH, 2), np.arange(1, DH, 2)])

    in_maps = []
    for c in range(NCORES):
        cols = np.concatenate(
            [np.arange(h * DH, (h + 1) * DH) for h in range(c, N_HEADS, N_KV)]
        )
        wq_c = wq[:, cols].reshape(DIM, HPC, DH)[:, :, perm].reshape(DIM, -1)
        wk_c = wk[:, c * DH:(c + 1) * DH][:, perm]
        wv_c = wv[:, c * DH:(c + 1) * DH]
        pv_, pk_, pq_ = (
            _ptile(wv_c).reshape(128, NDT, DH),
            _ptile(wk_c).reshape(128, NDT, DH),
            _ptile(wq_c).reshape(128, NDT, HPC * DH),
        )
        wall = np.concatenate([pv_, pk_, pq_], axis=2).reshape(128, -1)
        in_maps.append({
            "xt4": xt4,
            "wall": np.ascontiguousarray(wall),
            "wo": _ptile(wo[cols, :]),
            "cosk": cosk,
            "sink": sink,
            "dmask4": dmask4,
        })
    return in_maps


def kernel(x, mask, wq, wk, wv, wo):
    x = np.asarray(x, dtype=np.float32)
    mask = np.asarray(mask, dtype=np.float32)
    wq = np.asarray(wq, dtype=np.float32)
    wk = np.asarray(wk, dtype=np.float32)
    wv = np.asarray(wv, dtype=np.float32)
    wo = np.asarray(wo, dtype=np.float32)

    nc = _build()
    in_maps = _host_inputs(x, mask, wq, wk, wv, wo)

    res = run_bass_kernel_spmd(
        nc, in_maps, core_ids=list(range(NCORES)), trace=TRACE
    )
    LAST_RESULT[0] = res
    outs = res.results
    total = np.zeros((DIM, T), dtype=np.float32)
    for c in range(NCORES):
        total += np.asarray(outs[c]["out"], dtype=np.float32)
    return np.ascontiguousarray(total.T).reshape(B, L, DIM)


# revision 34
# speedup vs baseline: 1.0143x; 1.0143x over previous
"""GQA attention (B=4, L=1024, D=4096, 32 Q heads / 8 KV heads, head_dim=128,
traditional RoPE, causal mask) on 8 TRN2 NeuronCores.

Sharding: tensor-parallel over heads. Core c owns Q heads {c, c+8, c+16, c+24}
(all map to KV head c) - each core needs exactly one KV head. wq/wk/wv
column-sharded, wo row-sharded, x replicated. Each core computes a partial
output through wo; the host sums the 8 partials (and transposes: the kernel
writes out^T [DIM, T] in bf16).

v3 changes vs v2 (measured ~697-700us vs ~706-712us baseline, trace on):
- Attention uses 128-token q chunks x 4-head-wide score/PV matmuls (N=512).
  Finer causal blocking: 36 instead of 40 block-columns per batch. The
  softmax denominator comes from per-group pair-sums + a pairwise tree on
  DVE and ONE ones-matmul per q-chunk, cutting ~25k PE cycles net. (An
  earlier variant chained the sums on GpSimd - its serial ~1.2us/add chain
  starved the PE; keep this work on DVE, tree-shaped.)
- v^T produced by the DMA xbar transpose (dma_start_transpose) instead of PE
  matmul-transposes: no identity operand, no PSUM transpose bank.
- One unified 8-bank PSUM pool (4 slots x 2 banks, tags sc/pvden) shared by
  the QKV stage, attention and the output projection - no pool-boundary
  barriers between stages, slots rotate straight across A->B(b)->C(b)->B(b+1),
  and batch-0 attention is emitted directly behind the last QKV chunk.
- x is host-packed [p, tci, d, t] so each x DMA moves 4 d-slices with one
  contiguous 4KB segment per partition; prefetch runs a fixed 2 groups
  (8 d-slices) ahead - shallower prefetch measurably stalls stage A once
  per tci. wo streams in 4x1MB chunks (tci 1-4) so no single weight burst
  starves the x stream. RoPE q reuses the k cos/sin tables (same positional
  slices) with per-head muls, saving 20KB/partition of SBUF.
- Output projection writes alternate between the sync and scalar DMA rings,
  halving the end-of-kernel drain.
"""

import numpy as np
import ml_dtypes
from contextlib import ExitStack

import concourse.bass as bass
import concourse.mybir as mybir
import concourse.tile as tile
from concourse import bacc
from concourse.bass_utils import run_bass_kernel_spmd

DIM = 4096
N_HEADS = 32
N_KV = 8
DH = 128
B, L = 4, 1024
NCORES = 8
HPC = N_HEADS // NCORES  # 4 q-heads per core
T = B * L  # 4096 tokens total
SCALE = DH ** -0.5
ROPE_BASE = 10000.0
NDT = DIM // 128  # 32 contraction tiles

BF = mybir.dt.bfloat16
F32 = mybir.dt.float32
NPBF = ml_dtypes.bfloat16

QC = L // 128  # 8 q-chunks of 128 per batch
KT = L // 128  # 8 k tiles of 128 per batch
NTCI = T // 512  # 8 chunks of 512 tokens

TRACE = False
LAST_RESULT = [None]
DEBUG_STOP = None  # "B0" stops the build after batch-0 attention (sim debug)


def _check_mask(mask):
    """Verify the mask is the binary causal mask this kernel is specialized
    to."""
    m = np.asarray(mask)
    assert m.shape == (L, L)
    assert np.all((m == 0.0) | (m <= -1e8)), "kernel assumes binary additive mask"
    keep = (m == 0.0)  # [q, k]
    expect = np.tril(np.ones((L, L), dtype=bool))
    assert np.array_equal(keep, expect), "kernel assumes causal mask"


def _build():
    nc = bacc.Bacc(
        "TRN2", target_bir_lowering=False, debug=False, num_devices=NCORES
    )

    # x host-packed: [p, tci, d, t] so a 4-d-slice chunk is one contiguous
    # 4KB-per-partition DMA
    xt4 = nc.dram_tensor("xt4", [128, NTCI * NDT * 512], BF, kind="ExternalInput").ap()
    # all QKV weights in one tensor, d-major: [:, d] = [wv_d | wk_d | wq_d]
    wall = nc.dram_tensor("wall", [128, NDT * 768], BF, kind="ExternalInput").ap()
    wo = nc.dram_tensor("wo", [128, HPC * DIM], BF, kind="ExternalInput").ap()
    cosq = sinq = None
    cosk = nc.dram_tensor("cosk", [128, L], BF, kind="ExternalInput").ap()
    sink = nc.dram_tensor("sink", [128, L], BF, kind="ExternalInput").ap()
    dmask4 = nc.dram_tensor("dmask4", [128, HPC * 128], BF, kind="ExternalInput").ap()
    out = nc.dram_tensor("out", [DIM, T], BF, kind="ExternalOutput").ap()

    xt4_r = xt4.rearrange("p (tci d t) -> p tci d t", tci=NTCI, d=NDT)
    wall_r = wall.rearrange("p (dt m) -> p dt m", dt=NDT)  # [128, 32, 768]
    del cosq, sinq  # q rope shares the k tables (same positional slices)

    with tile.TileContext(nc) as tc, ExitStack() as ctx:
        persist = ctx.enter_context(tc.tile_pool(name="persist", bufs=1))
        qt_pool = ctx.enter_context(tc.tile_pool(name="qt", bufs=B))
        kt_pool = ctx.enter_context(tc.tile_pool(name="kt", bufs=B))
        v_pool = ctx.enter_context(tc.tile_pool(name="v", bufs=B))
        wo_p = ctx.enter_context(tc.tile_pool(name="wo_p", bufs=1))
        # unified PSUM pool: 4 slots x 2 banks, tags "sc" and "pvden"
        psU = ctx.enter_context(tc.tile_pool(name="psU", bufs=2, space="PSUM"))
        ones_sb = persist.tile([128, 128], BF)
        nc.vector.memset(ones_sb, 1.0)
        dmsk_sb = persist.tile([128, 128], BF)
        wo_sb = wo_p.tile([128, HPC, DIM], BF)

        qt_all = [None] * B  # [128 dh, HPC, 1024] rope'd q, halves layout
        kt_all = [None] * B  # [128 dh, 1024]
        v_t = [None] * B     # [128 t, KT, 128 dh]

        # ---------------- Stage A: QKV projection + RoPE ----------------
        with tc.tile_pool(name="wA", bufs=1) as wA, \
             tc.tile_pool(name="xp", bufs=3) as xp, \
             tc.tile_pool(name="evq", bufs=2) as evq, \
             tc.tile_pool(name="evs", bufs=2) as evs, \
             tc.tile_pool(name="rtmp", bufs=1) as rtmp:

            wall_sb = wA.tile([128, NDT, 768], BF)
            cosk_sb = wA.tile([128, L], BF)
            sink_sb = wA.tile([128, L], BF)

            def _wv(d):
                return wall_sb[:, d, 0:DH]

            def _wk(d):
                return wall_sb[:, d, DH:2 * DH]

            def _wq(d, h):
                return wall_sb[:, d, 2 * DH + h * DH:2 * DH + (h + 1) * DH]

            # weight chunks of 4 d-slices on the scalar ring, concurrent with
            # the x stream on the sync ring
            def _wchunk(k):
                dsl = slice(4 * k, 4 * k + 4)
                nc.scalar.dma_start(out=wall_sb[:, dsl], in_=wall_r[:, dsl])

            # x group g = (tci, j): 4 d-slices [128, 4, 512]
            xtiles = {}

            def _xfetch(g):
                tci_g, j = divmod(g, 8)
                xt = xp.tile([128, 4, 512], BF, name="xt4t")
                nc.sync.dma_start(
                    out=xt, in_=xt4_r[:, tci_g, 4 * j:4 * j + 4, :]
                )
                xtiles[g] = xt

            nc.scalar.dma_start(out=wall_sb[:, 0:1], in_=wall_r[:, 0:1])
            _xfetch(0)
            nc.scalar.dma_start(out=wall_sb[:, 1:4], in_=wall_r[:, 1:4])
            _xfetch(1)
            _wchunk(1)
            _xfetch(2)

            # HAM warmup: keep the PE busy while the first DMAs land
            wu = psU.tile([128, 2, 512], F32, name="wu", tag="sc")
            for _ in range(40):
                nc.tensor.matmul(
                    wu[:, 0, 0:128], ones_sb, ones_sb, start=True, stop=True
                )

            for tci in range(NTCI):
                b, half = tci // 2, tci % 2
                lsl = slice(half * 512, (half + 1) * 512)
                if half == 0:
                    qt_all[b] = qt_pool.tile([128, HPC, L], BF, name="qtile")
                    kt_all[b] = kt_pool.tile([128, L], BF, name="ktile")
                    v_t[b] = v_pool.tile([128, KT, DH], BF, name="vtile")

                vk = psU.tile([128, 2, 512], F32, name="vk", tag="sc")
                q01 = psU.tile([128, 2, 512], F32, name="q01", tag="sc")
                q23 = psU.tile([128, 2, 512], F32, name="q23", tag="pvden")

                for d in range(NDT):
                    if tci == 0:
                        if d % 4 == 1 and d // 4 + 2 < 8:
                            _wchunk(d // 4 + 2)
                        if d == 8:
                            nc.scalar.dma_start(
                                out=dmsk_sb, in_=dmask4[:, 0:128]
                            )
                        if d == 20:
                            nc.scalar.dma_start(out=cosk_sb, in_=cosk)
                        if d == 23:
                            nc.scalar.dma_start(out=sink_sb, in_=sink)
                    if 1 <= tci <= 4 and d == 16:
                        # wo in 4x1MB chunks so no burst starves the x stream
                        h = tci - 1
                        nc.scalar.dma_start(
                            out=wo_sb[:, h],
                            in_=wo[:, h * DIM:(h + 1) * DIM],
                        )
                    if d % 4 == 0:
                        g = tci * 8 + d // 4
                        if g + 2 < NTCI * 8 and g + 2 not in xtiles:
                            _xfetch(g + 2)
                        xtiles.pop(g - 1, None)  # free previous group ref
                        cur = xtiles[g]
                    xt = cur[:, d % 4]
                    st, sp = d == 0, d == NDT - 1
                    nc.tensor.matmul(vk[:, 0], _wv(d), xt, start=st, stop=sp)
                    nc.tensor.matmul(vk[:, 1], _wk(d), xt, start=st, stop=sp)
                    nc.tensor.matmul(q01[:, 0], _wq(d, 0), xt, start=st, stop=sp)
                    nc.tensor.matmul(q01[:, 1], _wq(d, 1), xt, start=st, stop=sp)
                    nc.tensor.matmul(q23[:, 0], _wq(d, 2), xt, start=st, stop=sp)
                    nc.tensor.matmul(q23[:, 1], _wq(d, 3), xt, start=st, stop=sp)

                # --- tail: evacuate + v-transpose (DMA xbar) + RoPE ---
                vraw = evs.tile([128, 512], BF, name="vraw")
                nc.scalar.copy(vraw, vk[:, 0])
                nc.sync.dma_start_transpose(
                    out=v_t[b][:, half * 4:(half + 1) * 4, :], in_=vraw
                )

                kraw = evs.tile([128, 512], BF, name="kraw")
                nc.vector.tensor_copy(kraw, vk[:, 1])
                qraw = evq.tile([128, HPC, 512], BF, name="qraw")
                nc.scalar.copy(qraw[:, 0:2], q01)
                nc.vector.tensor_copy(qraw[:, 2:4], q23)

                # RoPE: dst = raw*cos + swap64(raw*sinSw)  (halves layout)
                u_k = rtmp.tile([128, 512], BF, name="uk")
                t_k = rtmp.tile([128, 512], BF, name="tk")
                usw_k = rtmp.tile([128, 512], BF, name="uswk")
                nc.vector.tensor_mul(u_k, kraw, sink_sb[:, lsl])
                nc.vector.tensor_mul(t_k, kraw, cosk_sb[:, lsl])
                nc.vector.tensor_copy(usw_k[0:64], u_k[64:128])
                nc.vector.tensor_copy(usw_k[64:128], u_k[0:64])
                nc.vector.tensor_add(kt_all[b][:, lsl], t_k, usw_k)

                u_q = rtmp.tile([128, HPC, 512], BF, name="uq")
                t_q = rtmp.tile([128, HPC, 512], BF, name="tq")
                usw_q = rtmp.tile([128, HPC, 512], BF, name="uswq")
                for h in range(HPC):
                    nc.vector.tensor_mul(u_q[:, h], qraw[:, h], sink_sb[:, lsl])
                    nc.vector.tensor_mul(t_q[:, h], qraw[:, h], cosk_sb[:, lsl])
                nc.vector.tensor_copy(usw_q[0:64], u_q[64:128])
                nc.vector.tensor_copy(usw_q[64:128], u_q[0:64])
                nc.vector.tensor_add(qt_all[b][:, :, lsl], t_q, usw_q)

        # ---------------- Stages B + C, per batch ----------------
        attn_pool = ctx.enter_context(tc.tile_pool(name="attn", bufs=2))
        ep = ctx.enter_context(tc.tile_pool(name="ep", bufs=4))
        e2p = ctx.enter_context(tc.tile_pool(name="e2p", bufs=8))
        rcp = ctx.enter_context(tc.tile_pool(name="rcp", bufs=2))
        oev = ctx.enter_context(tc.tile_pool(name="oev", bufs=4))
        attn_all = [None] * B

        def gen_B(b):
            """Emit batch-b attention one q-chunk chain per next()."""
            attn_all[b] = attn_pool.tile([128, HPC, L], BF, name="atile")

            # ---- B(b): attention. 128-token q chunks, 4 heads wide.
            # Software-pipelined one score-group ahead: the PE always has the
            # next group's score matmuls queued while ACT runs exp and DVE
            # accumulates the softmax denominator.
            pvden_cur = [None]  # pv/den psum of the current q chunk
            e2s_cur = [[]]      # per-group pair-sums of e for the denominator
            pending = [None]

            def flush_pv(item):
                qc, g, e_g, first, last, e2s = item
                if first:
                    pvden_cur[0] = psU.tile(
                        [128, 2, 512], F32, name="pvden", tag="pvden"
                    )
                pvden = pvden_cur[0]
                for i, kt in enumerate(g):
                    nc.tensor.matmul(
                        pvden[:, 0], v_t[b][:, kt], e_g[:, i],
                        start=(first and i == 0), stop=(last and i == len(g) - 1),
                    )
                if last:
                    # DVE pairwise tree over the group sums, then one
                    # ones-matmul gives the softmax denominator
                    lvl = list(e2s)
                    while len(lvl) > 1:
                        nxt = []
                        for j in range(0, len(lvl) - 1, 2):
                            t_ = e2p.tile([128, 512], BF, name="e4")
                            nc.vector.tensor_add(t_, lvl[j], lvl[j + 1])
                            nxt.append(t_)
                        if len(lvl) % 2:
                            nxt.append(lvl[-1])
                        lvl = nxt
                    nc.tensor.matmul(
                        pvden[:, 1], ones_sb, lvl[0],
                        start=True, stop=True,
                    )
                    recip = rcp.tile([128, 512], F32, name="recip")
                    nc.vector.reciprocal_approx_fast(recip, pvden[:, 1])
                    q_sl = slice(qc * 128, (qc + 1) * 128)
                    nc.vector.tensor_mul(
                        attn_all[b][:, :, q_sl],
                        pvden[:, 0].rearrange("p (h q) -> p h q", h=HPC),
                        recip.rearrange("p (h q) -> p h q", h=HPC),
                    )

            for qc in range(QC):
                kts = list(range(qc + 1))
                groups = [kts[i:i + 2] for i in range(0, len(kts), 2)]
                q_ap = qt_all[b][:, :, qc * 128:(qc + 1) * 128]  # [128,4,128]
                for gi, g in enumerate(groups):
                    s = len(g)
                    first, last = gi == 0, gi == len(groups) - 1
                    sc = psU.tile(
                        [128, 2, HPC, 128], F32, name="sc", tag="sc",
                    )
                    for i, kt in enumerate(g):
                        nc.tensor.matmul(
                            sc[:, i],
                            kt_all[b][:, kt * 128:(kt + 1) * 128],
                            q_ap,
                            start=True, stop=True,
                        )
                    e_g = ep.tile([128, 2, HPC, 128], BF, name="etile")
                    nc.scalar.activation(
                        e_g[:, 0:s], sc[:, 0:s],
                        mybir.ActivationFunctionType.Exp,
                        scale=SCALE,
                    )
                    if last:
                        # diagonal block: causal keep-pattern, same for all qc
                        for h_ in range(HPC):
                            nc.vector.tensor_mul(
                                e_g[:, s - 1, h_], e_g[:, s - 1, h_], dmsk_sb
                            )
                    # per-group pair-sum for the denominator (DVE, parallel)
                    if first:
                        e2s_cur[0] = []
                    if s == 2:
                        e2 = e2p.tile([128, 512], BF, name="e2")
                        nc.vector.tensor_add(
                            e2, e_g[:, 0].rearrange("p h q -> p (h q)"),
                            e_g[:, 1].rearrange("p h q -> p (h q)"),
                        )
                    else:
                        e2 = e_g[:, 0].rearrange("p h q -> p (h q)")
                    e2s_cur[0].append(e2)
                    if pending[0] is not None:
                        flush_pv(pending[0])
                    pending[0] = (qc, g, e_g, first, last, list(e2s_cur[0]))
                yield qc
            flush_pv(pending[0])
            pending[0] = None
            yield QC

        gens = [gen_B(b_) for b_ in range(B)]

        def pump(gen, n=None):
            took = 0
            for _ in gen:
                took += 1
                if n is not None and took >= n:
                    return

        pump(gens[0])  # B(0) in full, right on the heels of stage A

        if DEBUG_STOP != "B0":
            for b in range(B):
                # ---- C(b): output projection, wo stationary, out^T ----
                for nb in range(DIM // 128):  # 32 blocks of 128 output cols
                    ps_c = psU.tile(
                        [128, 2, 512], F32, name="psc",
                        tag=("sc" if nb % 2 == 0 else "pvden"),
                    )
                    for h in range(HPC):
                        for t2 in range(2):
                            nc.tensor.matmul(
                                ps_c[:, t2],
                                wo_sb[:, h, nb * 128:(nb + 1) * 128],
                                attn_all[b][:, h, t2 * 512:(t2 + 1) * 512],
                                start=(h == 0), stop=(h == HPC - 1),
                            )
                    o_sb = oev.tile([128, 1024], BF, name="osb")
                    if nb % 2 == 0:
                        nc.vector.tensor_copy(o_sb, ps_c)
                        eng = nc.sync
                    else:
                        nc.scalar.copy(o_sb, ps_c)
                        eng = nc.scalar
                    eng.dma_start(
                        out=out[nb * 128:(nb + 1) * 128, b * L:(b + 1) * L],
                        in_=o_sb,
                    )
                if b + 1 < B:
                    pump(gens[b + 1])  # rest of B(b+1)

    nc.finalize()
    return nc


def _host_tables():
    """cos/sin tables in the halves layout: row i (i<64) = even dim 2i,
    row 64+i = odd dim 2i+1. u = raw*sinSw; dst_lo = t1_lo + u_hi needs
    sinSw = [+sin; -sin]; cosH = [cos; cos]."""
    inv = ROPE_BASE ** (-np.arange(0, DH, 2, dtype=np.float64) / DH)  # [64]
    pos = np.arange(L, dtype=np.float64)
    ang = inv[:, None] * pos[None, :]  # [64, L]
    cosA, sinA = np.cos(ang), np.sin(ang)
    cosH = np.concatenate([cosA, cosA], axis=0)  # [128, L]
    sinSw = np.concatenate([sinA, -sinA], axis=0)  # [128, L]
    cosq = np.empty((128, 2, HPC, 512), dtype=np.float64)
    sinq = np.empty((128, 2, HPC, 512), dtype=np.float64)
    for half in range(2):
        sl = slice(half * 512, (half + 1) * 512)
        cosq[:, half] = cosH[:, sl][:, None, :]
        sinq[:, half] = sinSw[:, sl][:, None, :]
    return (
        cosq.reshape(128, -1).astype(NPBF),
        sinq.reshape(128, -1).astype(NPBF),
        np.ascontiguousarray(cosH).astype(NPBF),
        np.ascontiguousarray(sinSw).astype(NPBF),
    )


def _ptile(w):
    # [K, M] -> partition-major [128, (K/128, M)] host pre-tiling
    k, m = w.shape
    return np.ascontiguousarray(
        w.reshape(k // 128, 128, m).transpose(1, 0, 2).reshape(128, -1)
    ).astype(NPBF)


def _host_inputs(x, mask, wq, wk, wv, wo):
    _check_mask(mask)

    xT = np.ascontiguousarray(x.reshape(T, DIM).T).astype(NPBF)
    # packed [p, tci, d, t]
    xt4 = np.ascontiguousarray(
        xT.reshape(NDT, 128, NTCI, 512).transpose(1, 2, 0, 3).reshape(128, -1)
    )
    _, _, cosk, sink = _host_tables()
    # diagonal causal keep-pattern [k, q] = (q >= k), replicated x4 heads
    tri = np.triu(np.ones((128, 128), dtype=np.float32))
    dmask4 = np.ascontiguousarray(
        np.tile(tri[:, None, :], (1, HPC, 1)).reshape(128, -1)
    ).astype(NPBF)

    # halves permutation of the head_dim axis: even dims then odd dims
    perm = np.concatenate([np.arange(0, DH, 2), np.arange(1, DH, 2)])

    in_maps = []
    for c in range(NCORES):
        cols = np.concatenate(
            [np.arange(h * DH, (h + 1) * DH) for h in range(c, N_HEADS, N_KV)]
        )
        wq_c = wq[:, cols].reshape(DIM, HPC, DH)[:, :, perm].reshape(DIM, -1)
        wk_c = wk[:, c * DH:(c + 1) * DH][:, perm]
        wv_c = wv[:, c * DH:(c + 1) * DH]
        pv_, pk_, pq_ = (
            _ptile(wv_c).reshape(128, NDT, DH),
            _ptile(wk_c).reshape(128, NDT, DH),
            _ptile(wq_c).reshape(128, NDT, HPC * DH),
        )
        wall = np.concatenate([pv_, pk_, pq_], axis=2).reshape(128, -1)
        in_maps.append({
            "xt4": xt4,
            "wall": np.ascontiguousarray(wall),
            "wo": _ptile(wo[cols, :]),
            "cosk": cosk,
            "sink": sink,
            "dmask4": dmask4,
        })
    return in_maps


def kernel(x, mask, wq, wk, wv, wo):
    x = np.asarray(x, dtype=np.float32)
    mask = np.asarray(mask, dtype=np.float32)
    wq = np.asarray(wq, dtype=np.float32)
    wk = np.asarray(wk, dtype=np.float32)
    wv = np.asarray(wv, dtype=np.float32)
    wo = np.asarray(wo, dtype=np.float32)

    nc = _build()
    in_maps = _host_inputs(x, mask, wq, wk, wv, wo)

    res = run_bass_kernel_spmd(
        nc, in_maps, core_ids=list(range(NCORES)), trace=TRACE
    )
    LAST_RESULT[0] = res
    outs = res.results
    total = np.zeros((DIM, T), dtype=np.float32)
    for c in range(NCORES):
        total += np.asarray(outs[c]["out"], dtype=np.float32)
    return np.ascontiguousarray(total.T).reshape(B, L, DIM)


# revision 35
# speedup vs baseline: 1.0274x; 1.0129x over previous
"""GQA attention (B=4, L=1024, D=4096, 32 Q heads / 8 KV heads, head_dim=128,
traditional RoPE, causal mask) on 8 TRN2 NeuronCores.

Sharding: tensor-parallel over heads. Core c owns Q heads {c, c+8, c+16, c+24}
(all map to KV head c) - each core needs exactly one KV head. wq/wk/wv
column-sharded, wo row-sharded, x replicated. Each core computes a partial
output through wo; the host sums the 8 partials (and transposes: the kernel
writes out^T [DIM, T] in bf16).

v3 changes vs v2 (measured ~697-700us vs ~706-712us baseline, trace on):
- Attention uses 128-token q chunks x 4-head-wide score/PV matmuls (N=512).
  Finer causal blocking: 36 instead of 40 block-columns per batch. The
  softmax denominator comes from per-group pair-sums + a pairwise tree on
  DVE and ONE ones-matmul per q-chunk, cutting ~25k PE cycles net. (An
  earlier variant chained the sums on GpSimd - its serial ~1.2us/add chain
  starved the PE; keep this work on DVE, tree-shaped.)
- v^T produced by the DMA xbar transpose (dma_start_transpose) instead of PE
  matmul-transposes: no identity operand, no PSUM transpose bank.
- One unified 8-bank PSUM pool (4 slots x 2 banks, tags sc/pvden) shared by
  the QKV stage, attention and the output projection - no pool-boundary
  barriers between stages, slots rotate straight across A->B(b)->C(b)->B(b+1),
  and batch-0 attention is emitted directly behind the last QKV chunk.
- x is host-packed [p, tci, d, t] so each x DMA moves 4 d-slices with one
  contiguous 4KB segment per partition; prefetch runs a fixed 2 groups
  (8 d-slices) ahead - shallower prefetch measurably stalls stage A once
  per tci. wo streams in 4x1MB chunks (tci 1-4) so no single weight burst
  starves the x stream. RoPE q reuses the k cos/sin tables (same positional
  slices) with per-head muls, saving 20KB/partition of SBUF.
- Output projection writes alternate between the sync and scalar DMA rings,
  halving the end-of-kernel drain.
"""

import numpy as np
import ml_dtypes
from contextlib import ExitStack

import concourse.bass as bass
import concourse.mybir as mybir
import concourse.tile as tile
from concourse import bacc
from concourse.bass_utils import run_bass_kernel_spmd

DIM = 4096
N_HEADS = 32
N_KV = 8
DH = 128
B, L = 4, 1024
NCORES = 8
HPC = N_HEADS // NCORES  # 4 q-heads per core
T = B * L  # 4096 tokens total
SCALE = DH ** -0.5
ROPE_BASE = 10000.0
NDT = DIM // 128  # 32 contraction tiles

BF = mybir.dt.bfloat16
F32 = mybir.dt.float32
NPBF = ml_dtypes.bfloat16

QC = L // 128  # 8 q-chunks of 128 per batch
KT = L // 128  # 8 k tiles of 128 per batch
NTCI = T // 512  # 8 chunks of 512 tokens

TRACE = False
LAST_RESULT = [None]
DEBUG_STOP = None  # "B0" stops the build after batch-0 attention (sim debug)


def _check_mask(mask):
    """Verify the mask is the binary causal mask this kernel is specialized
    to."""
    m = np.asarray(mask)
    assert m.shape == (L, L)
    assert np.all((m == 0.0) | (m <= -1e8)), "kernel assumes binary additive mask"
    keep = (m == 0.0)  # [q, k]
    expect = np.tril(np.ones((L, L), dtype=bool))
    assert np.array_equal(keep, expect), "kernel assumes causal mask"


def _build():
    nc = bacc.Bacc(
        "TRN2", target_bir_lowering=False, debug=False, num_devices=NCORES
    )

    # x host-packed: [p, tci, d, t] so a 4-d-slice chunk is one contiguous
    # 4KB-per-partition DMA
    xt4 = nc.dram_tensor("xt4", [128, NTCI * NDT * 512], BF, kind="ExternalInput").ap()
    # all QKV weights in one tensor, d-major: [:, d] = [wv_d | wk_d | wq_d]
    wall = nc.dram_tensor("wall", [128, NDT * 768], BF, kind="ExternalInput").ap()
    wo = nc.dram_tensor("wo", [128, HPC * DIM], BF, kind="ExternalInput").ap()
    cosq = sinq = None
    cosk = nc.dram_tensor("cosk", [128, L], BF, kind="ExternalInput").ap()
    sink = nc.dram_tensor("sink", [128, L], BF, kind="ExternalInput").ap()
    dmask4 = nc.dram_tensor("dmask4", [128, HPC * 128], BF, kind="ExternalInput").ap()
    out = nc.dram_tensor("out", [DIM, T], BF, kind="ExternalOutput").ap()

    xt4_r = xt4.rearrange("p (tci d t) -> p tci d t", tci=NTCI, d=NDT)
    wall_r = wall.rearrange("p (dt m) -> p dt m", dt=NDT)  # [128, 32, 768]
    del cosq, sinq  # q rope shares the k tables (same positional slices)

    with tile.TileContext(nc) as tc, ExitStack() as ctx:
        persist = ctx.enter_context(tc.tile_pool(name="persist", bufs=1))
        qt_pool = ctx.enter_context(tc.tile_pool(name="qt", bufs=B))
        kt_pool = ctx.enter_context(tc.tile_pool(name="kt", bufs=B))
        v_pool = ctx.enter_context(tc.tile_pool(name="v", bufs=B))
        wo_p = ctx.enter_context(tc.tile_pool(name="wo_p", bufs=1))
        # unified PSUM pool: 4 slots x 2 banks, tags "sc" and "pvden"
        psU = ctx.enter_context(tc.tile_pool(name="psU", bufs=2, space="PSUM"))
        ones_sb = persist.tile([128, 128], BF)
        nc.vector.memset(ones_sb, 1.0)
        dmsk_sb = persist.tile([128, 128], BF)
        wo_sb = wo_p.tile([128, HPC, DIM], BF)

        qt_all = [None] * B  # [128 dh, HPC, 1024] rope'd q, halves layout
        kt_all = [None] * B  # [128 dh, 1024]
        v_t = [None] * B     # [128 t, KT, 128 dh]

        # ---------------- Stage A: QKV projection + RoPE ----------------
        with tc.tile_pool(name="wA", bufs=1) as wA, \
             tc.tile_pool(name="xp", bufs=3) as xp, \
             tc.tile_pool(name="evq", bufs=2) as evq, \
             tc.tile_pool(name="evs", bufs=2) as evs, \
             tc.tile_pool(name="rtmp", bufs=1) as rtmp:

            wall_sb = wA.tile([128, NDT, 768], BF)
            cosk_sb = wA.tile([128, L], BF)
            sink_sb = wA.tile([128, L], BF)

            def _wv(d):
                return wall_sb[:, d, 0:DH]

            def _wk(d):
                return wall_sb[:, d, DH:2 * DH]

            def _wq(d, h):
                return wall_sb[:, d, 2 * DH + h * DH:2 * DH + (h + 1) * DH]

            # weight chunks of 4 d-slices on the scalar ring, concurrent with
            # the x stream on the sync ring
            def _wchunk(k):
                dsl = slice(4 * k, 4 * k + 4)
                nc.scalar.dma_start(out=wall_sb[:, dsl], in_=wall_r[:, dsl])

            # x group g = (tci, j): 4 d-slices [128, 4, 512]
            xtiles = {}

            def _xfetch(g):
                tci_g, j = divmod(g, 8)
                xt = xp.tile([128, 4, 512], BF, name="xt4t")
                nc.sync.dma_start(
                    out=xt, in_=xt4_r[:, tci_g, 4 * j:4 * j + 4, :]
                )
                xtiles[g] = xt

            nc.scalar.dma_start(out=wall_sb[:, 0:1], in_=wall_r[:, 0:1])
            _xfetch(0)
            nc.scalar.dma_start(out=wall_sb[:, 1:4], in_=wall_r[:, 1:4])
            _xfetch(1)
            _wchunk(1)
            _xfetch(2)

            # HAM warmup: keep the PE busy while the first DMAs land
            wu = psU.tile([128, 2, 512], F32, name="wu", tag="sc")
            for _ in range(40):
                nc.tensor.matmul(
                    wu[:, 0, 0:128], ones_sb, ones_sb, start=True, stop=True
                )

            for tci in range(NTCI):
                b, half = tci // 2, tci % 2
                lsl = slice(half * 512, (half + 1) * 512)
                if half == 0:
                    qt_all[b] = qt_pool.tile([128, HPC, L], BF, name="qtile")
                    kt_all[b] = kt_pool.tile([128, L], BF, name="ktile")
                    v_t[b] = v_pool.tile([128, KT, DH], BF, name="vtile")

                vk = psU.tile([128, 2, 512], F32, name="vk", tag="sc")
                q01 = psU.tile([128, 2, 512], F32, name="q01", tag="sc")
                q23 = psU.tile([128, 2, 512], F32, name="q23", tag="pvden")

                for d in range(NDT):
                    if tci == 0:
                        if d % 4 == 1 and d // 4 + 2 < 8:
                            _wchunk(d // 4 + 2)
                        if d == 8:
                            nc.scalar.dma_start(
                                out=dmsk_sb, in_=dmask4[:, 0:128]
                            )
                        if d == 20:
                            nc.scalar.dma_start(out=cosk_sb, in_=cosk)
                        if d == 23:
                            nc.scalar.dma_start(out=sink_sb, in_=sink)
                    if 1 <= tci <= 4 and d == 16:
                        # wo in 4x1MB chunks so no burst starves the x stream
                        h = tci - 1
                        nc.scalar.dma_start(
                            out=wo_sb[:, h],
                            in_=wo[:, h * DIM:(h + 1) * DIM],
                        )
                    if d % 4 == 0:
                        g = tci * 8 + d // 4
                        if g + 2 < NTCI * 8 and g + 2 not in xtiles:
                            _xfetch(g + 2)
                        xtiles.pop(g - 1, None)  # free previous group ref
                        cur = xtiles[g]
                    xt = cur[:, d % 4]
                    st, sp = d == 0, d == NDT - 1
                    nc.tensor.matmul(vk[:, 0], _wv(d), xt, start=st, stop=sp)
                    nc.tensor.matmul(vk[:, 1], _wk(d), xt, start=st, stop=sp)
                    nc.tensor.matmul(q01[:, 0], _wq(d, 0), xt, start=st, stop=sp)
                    nc.tensor.matmul(q01[:, 1], _wq(d, 1), xt, start=st, stop=sp)
                    nc.tensor.matmul(q23[:, 0], _wq(d, 2), xt, start=st, stop=sp)
                    nc.tensor.matmul(q23[:, 1], _wq(d, 3), xt, start=st, stop=sp)

                # --- tail: evacuate + v-transpose (DMA xbar) + RoPE ---
                vraw = evs.tile([128, 512], BF, name="vraw")
                nc.scalar.copy(vraw, vk[:, 0])
                nc.sync.dma_start_transpose(
                    out=v_t[b][:, half * 4:(half + 1) * 4, :], in_=vraw
                )

                kraw = evs.tile([128, 512], BF, name="kraw")
                nc.vector.tensor_copy(kraw, vk[:, 1])
                qraw = evq.tile([128, HPC, 512], BF, name="qraw")
                nc.scalar.copy(qraw[:, 0:2], q01)
                nc.vector.tensor_copy(qraw[:, 2:4], q23)

                # RoPE: dst = raw*cos + swap64(raw*sinSw)  (halves layout)
                u_k = rtmp.tile([128, 512], BF, name="uk")
                t_k = rtmp.tile([128, 512], BF, name="tk")
                usw_k = rtmp.tile([128, 512], BF, name="uswk")
                nc.vector.tensor_mul(u_k, kraw, sink_sb[:, lsl])
                nc.vector.tensor_mul(t_k, kraw, cosk_sb[:, lsl])
                nc.vector.tensor_copy(usw_k[0:64], u_k[64:128])
                nc.vector.tensor_copy(usw_k[64:128], u_k[0:64])
                nc.vector.tensor_add(kt_all[b][:, lsl], t_k, usw_k)

                u_q = rtmp.tile([128, HPC, 512], BF, name="uq")
                t_q = rtmp.tile([128, HPC, 512], BF, name="tq")
                usw_q = rtmp.tile([128, HPC, 512], BF, name="uswq")
                for h in range(HPC):
                    nc.vector.tensor_mul(u_q[:, h], qraw[:, h], sink_sb[:, lsl])
                    nc.vector.tensor_mul(t_q[:, h], qraw[:, h], cosk_sb[:, lsl])
                nc.vector.tensor_copy(usw_q[0:64], u_q[64:128])
                nc.vector.tensor_copy(usw_q[64:128], u_q[0:64])
                nc.vector.tensor_add(qt_all[b][:, :, lsl], t_q, usw_q)

        # ---------------- Stages B + C, per batch ----------------
        attn_pool = ctx.enter_context(tc.tile_pool(name="attn", bufs=2))
        ep = ctx.enter_context(tc.tile_pool(name="ep", bufs=5))
        e2p = ctx.enter_context(tc.tile_pool(name="e2p", bufs=10))
        rcp = ctx.enter_context(tc.tile_pool(name="rcp", bufs=2))
        oev = ctx.enter_context(tc.tile_pool(name="oev", bufs=4))
        attn_all = [None] * B

        def gen_B(b):
            """Emit batch-b attention one q-chunk chain per next()."""
            attn_all[b] = attn_pool.tile([128, HPC, L], BF, name="atile")

            # ---- B(b): attention. 128-token q chunks, 4 heads wide.
            # Software-pipelined one score-group ahead: the PE always has the
            # next group's score matmuls queued while ACT runs exp and DVE
            # accumulates the softmax denominator.
            pvden_cur = [None]  # pv/den psum of the current q chunk
            e2s_cur = [[]]      # per-group pair-sums of e for the denominator
            pending = []        # up to 2 score-groups in flight ahead of PV

            def flush_pv(item):
                qc, g, e_g, first, last, e2s = item
                if first:
                    pvden_cur[0] = psU.tile(
                        [128, 2, 512], F32, name="pvden", tag="pvden"
                    )
                pvden = pvden_cur[0]
                for i, kt in enumerate(g):
                    nc.tensor.matmul(
                        pvden[:, 0], v_t[b][:, kt], e_g[:, i],
                        start=(first and i == 0), stop=(last and i == len(g) - 1),
                    )
                if last:
                    # DVE pairwise tree over the group sums, then one
                    # ones-matmul gives the softmax denominator
                    lvl = list(e2s)
                    while len(lvl) > 1:
                        nxt = []
                        for j in range(0, len(lvl) - 1, 2):
                            t_ = e2p.tile([128, 512], BF, name="e4")
                            nc.vector.tensor_add(t_, lvl[j], lvl[j + 1])
                            nxt.append(t_)
                        if len(lvl) % 2:
                            nxt.append(lvl[-1])
                        lvl = nxt
                    nc.tensor.matmul(
                        pvden[:, 1], ones_sb, lvl[0],
                        start=True, stop=True,
                    )
                    recip = rcp.tile([128, 512], F32, name="recip")
                    nc.vector.reciprocal_approx_fast(recip, pvden[:, 1])
                    q_sl = slice(qc * 128, (qc + 1) * 128)
                    nc.vector.tensor_mul(
                        attn_all[b][:, :, q_sl],
                        pvden[:, 0].rearrange("p (h q) -> p h q", h=HPC),
                        recip.rearrange("p (h q) -> p h q", h=HPC),
                    )

            for qc in range(QC):
                kts = list(range(qc + 1))
                groups = [kts[i:i + 2] for i in range(0, len(kts), 2)]
                q_ap = qt_all[b][:, :, qc * 128:(qc + 1) * 128]  # [128,4,128]
                for gi, g in enumerate(groups):
                    s = len(g)
                    first, last = gi == 0, gi == len(groups) - 1
                    sc = psU.tile(
                        [128, 2, HPC, 128], F32, name="sc", tag="sc",
                    )
                    for i, kt in enumerate(g):
                        nc.tensor.matmul(
                            sc[:, i],
                            kt_all[b][:, kt * 128:(kt + 1) * 128],
                            q_ap,
                            start=True, stop=True,
                        )
                    e_g = ep.tile([128, 2, HPC, 128], BF, name="etile")
                    nc.scalar.activation(
                        e_g[:, 0:s], sc[:, 0:s],
                        mybir.ActivationFunctionType.Exp,
                        scale=SCALE,
                    )
                    if last:
                        # diagonal block: causal keep-pattern, same for all qc
                        for h_ in range(HPC):
                            nc.vector.tensor_mul(
                                e_g[:, s - 1, h_], e_g[:, s - 1, h_], dmsk_sb
                            )
                    # per-group pair-sum for the denominator (DVE, parallel)
                    if first:
                        e2s_cur[0] = []
                    if s == 2:
                        e2 = e2p.tile([128, 512], BF, name="e2")
                        nc.vector.tensor_add(
                            e2, e_g[:, 0].rearrange("p h q -> p (h q)"),
                            e_g[:, 1].rearrange("p h q -> p (h q)"),
                        )
                    else:
                        e2 = e_g[:, 0].rearrange("p h q -> p (h q)")
                    e2s_cur[0].append(e2)
                    pending.append(
                        (qc, g, e_g, first, last, list(e2s_cur[0]))
                    )
                    if len(pending) > 2:
                        flush_pv(pending.pop(0))
                yield qc
            for it in pending:
                flush_pv(it)
            del pending[:]
            yield QC

        gens = [gen_B(b_) for b_ in range(B)]

        def pump(gen, n=None):
            took = 0
            for _ in gen:
                took += 1
                if n is not None and took >= n:
                    return

        pump(gens[0])  # B(0) in full, right on the heels of stage A

        if DEBUG_STOP != "B0":
            for b in range(B):
                # ---- C(b): output projection, wo stationary, out^T ----
                for nb in range(DIM // 128):  # 32 blocks of 128 output cols
                    ps_c = psU.tile(
                        [128, 2, 512], F32, name="psc",
                        tag=("sc" if nb % 2 == 0 else "pvden"),
                    )
                    for h in range(HPC):
                        for t2 in range(2):
                            nc.tensor.matmul(
                                ps_c[:, t2],
                                wo_sb[:, h, nb * 128:(nb + 1) * 128],
                                attn_all[b][:, h, t2 * 512:(t2 + 1) * 512],
                                start=(h == 0), stop=(h == HPC - 1),
                            )
                    o_sb = oev.tile([128, 1024], BF, name="osb")
                    if nb % 2 == 0:
                        nc.vector.tensor_copy(o_sb, ps_c)
                        eng = nc.sync
                    else:
                        nc.scalar.copy(o_sb, ps_c)
                        eng = nc.scalar
                    eng.dma_start(
                        out=out[nb * 128:(nb + 1) * 128, b * L:(b + 1) * L],
                        in_=o_sb,
                    )
                if b + 1 < B:
                    pump(gens[b + 1])  # rest of B(b+1)

    nc.finalize()
    return nc


def _host_tables():
    """cos/sin tables in the halves layout: row i (i<64) = even dim 2i,
    row 64+i = odd dim 2i+1. u = raw*sinSw; dst_lo = t1_lo + u_hi needs
    sinSw = [+sin; -sin]; cosH = [cos; cos]."""
    inv = ROPE_BASE ** (-np.arange(0, DH, 2, dtype=np.float64) / DH)  # [64]
    pos = np.arange(L, dtype=np.float64)
    ang = inv[:, None] * pos[None, :]  # [64, L]
    cosA, sinA = np.cos(ang), np.sin(ang)
    cosH = np.concatenate([cosA, cosA], axis=0)  # [128, L]
    sinSw = np.concatenate([sinA, -sinA], axis=0)  # [128, L]
    cosq = np.empty((128, 2, HPC, 512), dtype=np.float64)
    sinq = np.empty((128, 2, HPC, 512), dtype=np.float64)
    for half in range(2):
        sl = slice(half * 512, (half + 1) * 512)
        cosq[:, half] = cosH[:, sl][:, None, :]
        sinq[:, half] = sinSw[:, sl][:, None, :]
    return (
        cosq.reshape(128, -1).astype(NPBF),
        sinq.reshape(128, -1).astype(NPBF),
        np.ascontiguousarray(cosH).astype(NPBF),
        np.ascontiguousarray(sinSw).astype(NPBF),
    )


def _ptile(w):
    # [K, M] -> partition-major [128, (K/128, M)] host pre-tiling
    k, m = w.shape
    return np.ascontiguousarray(
        w.reshape(k // 128, 128, m).transpose(1, 0, 2).reshape(128, -1)
    ).astype(NPBF)


def _host_inputs(x, mask, wq, wk, wv, wo):
    _check_mask(mask)

    xT = np.ascontiguousarray(x.reshape(T, DIM).T).astype(NPBF)
    # packed [p, tci, d, t]
    xt4 = np.ascontiguousarray(
        xT.reshape(NDT, 128, NTCI, 512).transpose(1, 2, 0, 3).reshape(128, -1)
    )
    _, _, cosk, sink = _host_tables()
    # diagonal causal keep-pattern [k, q] = (q >= k), replicated x4 heads
    tri = np.triu(np.ones((128, 128), dtype=np.float32))
    dmask4 = np.ascontiguousarray(
        np.tile(tri[:, None, :], (1, HPC, 1)).reshape(128, -1)
    ).astype(NPBF)

    # halves permutation of the head_dim axis: even dims then odd dims
    perm = np.concatenate([np.arange(0, DH, 2), np.arange(1, DH, 2)])

    in_maps = []
    for c in range(NCORES):
        cols = np.concatenate(
            [np.arange(h * DH, (h + 1) * DH) for h in range(c, N_HEADS, N_KV)]
        )
        wq_c = wq[:, cols].reshape(DIM, HPC, DH)[:, :, perm].reshape(DIM, -1)
        wk_c = wk[:, c * DH:(c + 1) * DH][:, perm]
        wv_c = wv[:, c * DH:(c + 1) * DH]
        pv_, pk_, pq_ = (
            _ptile(wv_c).reshape(128, NDT, DH),
            _ptile(wk_c).reshape(128, NDT, DH),
            _ptile(wq_c).reshape(128, NDT, HPC * DH),
        )
        wall = np.concatenate([pv_, pk_, pq_], axis=2).reshape(128, -1)
        in_maps.append({
            "xt4": xt4,
            "wall": np.ascontiguousarray(wall),
            "wo": _ptile(wo[cols, :]),
            "cosk": cosk,
            "sink": sink,
            "dmask4": dmask4,
        })
    return in_maps


def kernel(x, mask, wq, wk, wv, wo):
    x = np.asarray(x, dtype=np.float32)
    mask = np.asarray(mask, dtype=np.float32)
    wq = np.asarray(wq, dtype=np.float32)
    wk = np.asarray(wk, dtype=np.float32)
    wv = np.asarray(wv, dtype=np.float32)
    wo = np.asarray(wo, dtype=np.float32)

    nc = _build()
    in_maps = _host_inputs(x, mask, wq, wk, wv, wo)

    res = run_bass_kernel_spmd(
        nc, in_maps, core_ids=list(range(NCORES)), trace=TRACE
    )
    LAST_RESULT[0] = res
    outs = res.results
    total = np.zeros((DIM, T), dtype=np.float32)
    for c in range(NCORES):
        total += np.asarray(outs[c]["out"], dtype=np.float32)
    return np.ascontiguousarray(total.T).reshape(B, L, DIM)
